# revision 22
# baseline (speedup 1.0000x reference)
# CHIEF attention-MIL pooling kernel for 8 TRN2 NeuronCores.
#
# Reference computation (N=50000, FEAT=2048, HID=1024, D_ATT=512):
#   h = relu(x @ W1 + b1)            [N, 1024]
#   a = tanh(h @ Wa + ba)            [N, 512]
#   g = sigmoid(h @ Wb + bb)         [N, 512]
#   s = (a*g) @ Wc + bc              [N, 1]
#   A = softmax(s.T)                 [1, N]
#   M = A @ h + relu(text @ Wt + bt) [1, 1024]
#   logits = M @ Wcls + bcls         [1, 2]
#
# Strategy: shard the instance dim N across 8 cores (data parallel). Each
# core computes, for its shard, the transposed activations hT=[hid, inst]
# (x is pre-transposed on the host so every matmul contracts over the
# partition dim with no on-device transposes), the gated-attention scores
# s, unnormalized softmax weights w = exp(s + bc) (s is O(1) here so no
# max-subtraction is needed for stability), and the weighted pooling
# partial p = sum_i w_i * h_i. The host then reduces the 8 partial
# (p, sum w) pairs and runs the tiny text/classifier tail in numpy.
#
# Matmuls run as float32r (fp32 storage, reduced-precision PE mode at
# 1 cycle/row for moving free dim >= 256, vs 4 cycles/row plain fp32).
# The BIR verifier requires every producer feeding an fp32r matmul to
# emit fp32r, so all matmul-facing DRAM params and SBUF tiles are
# declared float32r; non-matmul readers view them through .bitcast(f32).
# Padding instances are masked by adding -1e30 to their scores before
# exp (exp -> exact 0), via a per-core "maskbias" input.

import numpy as np
from contextlib import ExitStack

import concourse.bass as bass
import concourse.bacc as bacc
import concourse.mybir as mybir
import concourse.tile as tile
from concourse.bass_utils import run_bass_kernel_spmd

P = 128
S = 512              # instances per group (moving free dim of every matmul)
G = 13               # groups per core
NPC = G * S          # instances per core (6656, padded)
NCORES = 8
N = 50000
FEAT, HID, DATT, TXT, NCLS = 2048, 1024, 512, 768, 2
KF, JH, DC = FEAT // P, HID // P, DATT // P   # 16, 8, 4

f32 = mybir.dt.float32
f32r = mybir.dt.float32r
AF = mybir.ActivationFunctionType
ALU = mybir.AluOpType


def build_nc(groups=G):
    # Bacc (not bare Bass): its compile() runs move_matmul_waits_to_ldweights
    # and generate_event_semaphores, which legalize instructions down to the
    # 1-sync-wait hardware limit.
    nc = bacc.Bacc("TRN2", target_bir_lowering=False, debug=False)
    npc = groups * S

    xT = nc.dram_tensor("xT", [FEAT, npc], f32r, kind="ExternalInput").ap()
    W1 = nc.dram_tensor("W1", [FEAT, HID], f32r, kind="ExternalInput").ap()
    Wa = nc.dram_tensor("Wa", [HID, DATT], f32r, kind="ExternalInput").ap()
    Wb = nc.dram_tensor("Wb", [HID, DATT], f32r, kind="ExternalInput").ap()
    Wcp = nc.dram_tensor("Wcp", [P, DC], f32r, kind="ExternalInput").ap()
    b1p = nc.dram_tensor("b1p", [P, JH], f32, kind="ExternalInput").ap()
    bap = nc.dram_tensor("bap", [P, DC], f32, kind="ExternalInput").ap()
    bbp = nc.dram_tensor("bbp", [P, DC], f32, kind="ExternalInput").ap()
    bcp = nc.dram_tensor("bcp", [1, 1], f32, kind="ExternalInput").ap()
    # mask: 1.0 for real instances, 0.0 for padding (multiplies w)
    mask = nc.dram_tensor("mask", [1, npc], f32r, kind="ExternalInput").ap()
    onesv = nc.dram_tensor("onesv", [1, P], f32r, kind="ExternalInput").ap()

    p_out = nc.dram_tensor("p_out", [P, JH], f32, kind="ExternalOutput").ap()
    w_out = nc.dram_tensor("w_out", [1, npc], f32r, kind="ExternalOutput").ap()

    # A fused-weight-load (4-byte dtype) matmul can encode only ONE sync
    # wait on TRN2 ("Too many sync wait commands" in walrus otherwise).
    # Standalone ldweights (viewed as bf16 — ldweights rejects 4-byte
    # dtypes; the loaded bits are garbage but every real matmul reloads
    # its own weights) act as PE-side wait absorbers: they advance the
    # PE's observed semaphore clocks so the following matmuls need at
    # most one wait each.
    def absorb(ap):
        nc.tensor.ldweights(ap.bitcast(mybir.dt.bfloat16))

    with tile.TileContext(nc) as tc, ExitStack() as ctx:
        wp = ctx.enter_context(tc.tile_pool(name="weights", bufs=1))
        xp = ctx.enter_context(tc.tile_pool(name="x", bufs=4))
        hp = ctx.enter_context(tc.tile_pool(name="h", bufs=2))
        agp = ctx.enter_context(tc.tile_pool(name="ag", bufs=2))
        sp = ctx.enter_context(tc.tile_pool(name="scratch", bufs=2))
        pp = ctx.enter_context(tc.tile_pool(name="psum", bufs=8, space="PSUM"))

        # Resident weights/biases.
        W1s = wp.tile([P, KF, HID], f32r)
        for k in range(KF):
            nc.sync.dma_start(W1s[:, k, :], W1[k * P:(k + 1) * P, :])
        Was = wp.tile([P, JH, DATT], f32r)
        Wbs = wp.tile([P, JH, DATT], f32r)
        for j in range(JH):
            nc.sync.dma_start(Was[:, j, :], Wa[j * P:(j + 1) * P, :])
            nc.sync.dma_start(Wbs[:, j, :], Wb[j * P:(j + 1) * P, :])
        Wcs = wp.tile([P, DC], f32r)
        nc.sync.dma_start(Wcs[:], Wcp)
        b1s = wp.tile([P, JH], f32)
        nc.sync.dma_start(b1s[:], b1p)
        bas = wp.tile([P, DC], f32)
        nc.sync.dma_start(bas[:], bap)
        bbs = wp.tile([P, DC], f32)
        nc.sync.dma_start(bbs[:], bbp)
        bcs = wp.tile([1, 1], f32)
        nc.sync.dma_start(bcs[:], bcp)
        ones = wp.tile([1, P], f32r)
        nc.sync.dma_start(ones[:], onesv)

        # Absorb all weight-DMA waits into PE before the first real matmul.
        for k in range(KF):
            absorb(W1s[:, k, 0:1])
        for j in range(JH):
            absorb(Was[:, j, 0:1])
            absorb(Wbs[:, j, 0:1])
        absorb(Wcs[:, 0:1])
        absorb(ones[0:1, 0:1])

        pacc = wp.tile([P, JH * groups], f32)  # per-group pooling partials
        Psb = wp.tile([P, JH], f32)

        for g in range(groups):
            isl = slice(g * S, (g + 1) * S)

            # --- step 1: hT[j*P+p, i] = relu(sum_f W1[f, hid] xT[f, i] + b1)
            ph = [pp.tile([P, S], f32, tag="ps", name=f"ph{g}_{j}")
                  for j in range(JH)]
            for k in range(KF):
                xt = xp.tile([P, S], f32r, tag="xt")
                nc.sync.dma_start(xt[:], xT[k * P:(k + 1) * P, isl])
                if k == 0:
                    # j=0's matmul carries a PSUM-slot wait; absorb the
                    # xt DMA wait separately so it never needs two.
                    absorb(xt[:, 0:1])
                for j in range(JH):
                    nc.tensor.matmul(
                        ph[j][:], W1s[:, k, j * P:(j + 1) * P], xt[:],
                        start=(k == 0), stop=(k == KF - 1))
            hT = hp.tile([P, JH, S], f32r, tag="hT")
            for j in range(JH):
                nc.scalar.activation(hT[:, j, :], ph[j][:], AF.Relu,
                                     bias=b1s[:, j:j + 1])

            # --- steps 2/3: agT = tanh(.)*sigmoid(.) over d_att chunks
            agT = agp.tile([P, DC, S], f32r, tag="agT")
            for d in range(DC):
                pa = pp.tile([P, S], f32, tag="ps", name=f"pa{g}_{d}")
                pg = pp.tile([P, S], f32, tag="ps", name=f"pg{g}_{d}")
                for j in range(JH):
                    nc.tensor.matmul(
                        pa[:], Was[:, j, d * P:(d + 1) * P], hT[:, j, :],
                        start=(j == 0), stop=(j == JH - 1))
                for j in range(JH):
                    nc.tensor.matmul(
                        pg[:], Wbs[:, j, d * P:(d + 1) * P], hT[:, j, :],
                        start=(j == 0), stop=(j == JH - 1))
                at = sp.tile([P, S], f32r, tag="at")
                gt = sp.tile([P, S], f32r, tag="gt")
                nc.scalar.activation(at[:], pa[:], AF.Tanh, bias=bas[:, d:d + 1])
                nc.scalar.activation(gt[:], pg[:], AF.Sigmoid, bias=bbs[:, d:d + 1])
                nc.vector.tensor_mul(agT[:, d, :], at[:], gt[:])

            # --- step 4: s[1, i] = sum_d agT[d, i] Wc[d];
            #             w = exp(s + bc) * mask
            ps = pp.tile([1, S], f32, tag="ps", name=f"ps_s{g}")
            # s-matmul c=0 carries a PSUM-slot wait (ACT); absorb the
            # agT readiness wait (DVE) separately.
            absorb(agT[:, 0, 0:1])
            for c in range(DC):
                nc.tensor.matmul(ps[:], Wcs[:, c:c + 1], agT[:, c, :],
                                 start=(c == 0), stop=(c == DC - 1))
            mg = sp.tile([1, S], f32r, tag="mg")
            nc.sync.dma_start(mg[:], mask[:, isl])
            wg = sp.tile([1, S], f32r, tag="wg")
            nc.scalar.activation(wg[:], ps[0:1, :], AF.Exp, bias=bcs[:, 0:1])
            wgm = sp.tile([1, S], f32r, tag="wgm")
            nc.vector.tensor_mul(wgm[:], wg[:], mg[:])
            nc.sync.dma_start(w_out[:, isl], wgm[:])

            # --- step 5: pooling partial p_jg[p] = sum_i hT[p,j,i] * w[i]
            # Broadcast w across partitions with a K=1 PE matmul
            # (out[m, n] = ones[0, m] * w[0, n]); DVE reads it from PSUM.
            wB = pp.tile([P, S], f32, tag="ps", name=f"wB{g}")
            nc.tensor.matmul(wB[:], ones[:], wgm[:], start=True, stop=True)
            for j in range(JH):
                wh = sp.tile([P, S], f32, tag="wh")
                nc.vector.tensor_mul(wh[:], hT[:, j, :].bitcast(f32), wB[:])
                nc.vector.tensor_reduce(
                    pacc[:, j * groups + g:j * groups + g + 1], wh[:],
                    axis=mybir.AxisListType.X, op=ALU.add)

        nc.vector.tensor_reduce(
            Psb[:], pacc[:].rearrange("p (j g) -> p j g", g=groups),
            axis=mybir.AxisListType.X, op=ALU.add)
        nc.sync.dma_start(p_out, Psb[:])

    nc.compile()
    return nc


_CACHE = {}


def _get_nc():
    if "nc" not in _CACHE:
        _CACHE["nc"] = build_nc()
    return _CACHE["nc"]


def make_in_maps(x, W1, b1, Wa, ba, Wb, bb, Wc, bc):
    """Shard/pack the full inputs into per-core input maps."""
    x = np.ascontiguousarray(np.asarray(x, np.float32))
    ntot = NCORES * NPC
    xTp = np.zeros((FEAT, ntot), np.float32)
    xTp[:, :N] = x.T
    m = np.zeros(ntot, np.float32)
    m[:N] = 1.0

    W1 = np.ascontiguousarray(np.asarray(W1, np.float32))
    Wa = np.ascontiguousarray(np.asarray(Wa, np.float32))
    Wb = np.ascontiguousarray(np.asarray(Wb, np.float32))
    Wcp = np.ascontiguousarray(np.asarray(Wc, np.float32)[:, 0].reshape(DC, P).T)
    b1p = np.ascontiguousarray(np.asarray(b1, np.float32).reshape(JH, P).T)
    bap = np.ascontiguousarray(np.asarray(ba, np.float32).reshape(DC, P).T)
    bbp = np.ascontiguousarray(np.asarray(bb, np.float32).reshape(DC, P).T)
    bcp = np.asarray(bc, np.float32).reshape(1, 1)

    in_maps = []
    for c in range(NCORES):
        sl = slice(c * NPC, (c + 1) * NPC)
        in_maps.append({
            "xT": np.ascontiguousarray(xTp[:, sl]),
            "W1": W1, "Wa": Wa, "Wb": Wb, "Wcp": Wcp,
            "b1p": b1p, "bap": bap, "bbp": bbp, "bcp": bcp,
            "mask": np.ascontiguousarray(m[sl].reshape(1, NPC)),
            "onesv": np.ones((1, P), np.float32),
        })
    return in_maps


def finish(results, text_emb, Wt, bt, Wcls, bcls):
    """Host-side reduction of per-core partials + text/classifier tail."""
    p = np.zeros(HID, np.float64)
    z = 0.0
    for r in results:
        p += r["p_out"].astype(np.float64).T.reshape(-1)
        z += float(r["w_out"].sum(dtype=np.float64))
    M = (p / z).astype(np.float32)[None, :]                    # [1, 1024]
    text_emb = np.asarray(text_emb, np.float32)
    t = np.maximum(text_emb @ np.asarray(Wt, np.float32)
                   + np.asarray(bt, np.float32), 0.0)
    logits = (M + t) @ np.asarray(Wcls, np.float32) + np.asarray(bcls, np.float32)
    return logits.astype(np.float32)


def kernel(x, text_emb, W1, b1, Wa, ba, Wb, bb, Wc, bc, Wt, bt, Wcls, bcls,
           **kw):
    in_maps = make_in_maps(x, W1, b1, Wa, ba, Wb, bb, Wc, bc)
    res = run_bass_kernel_spmd(_get_nc(), in_maps, core_ids=list(range(NCORES)),
                               **kw)
    return finish(res.results, text_emb, Wt, bt, Wcls, bcls)


# revision 23
# speedup vs baseline: 1.0515x; 1.0515x over previous
# CHIEF attention-MIL pooling kernel for 8 TRN2 NeuronCores.
#
# Reference computation (N=50000, FEAT=2048, HID=1024, D_ATT=512):
#   h = relu(x @ W1 + b1)            [N, 1024]
#   a = tanh(h @ Wa + ba)            [N, 512]
#   g = sigmoid(h @ Wb + bb)         [N, 512]
#   s = (a*g) @ Wc + bc              [N, 1]
#   A = softmax(s.T)                 [1, N]
#   M = A @ h + relu(text @ Wt + bt) [1, 1024]
#   logits = M @ Wcls + bcls         [1, 2]
#
# Strategy: shard the instance dim N across 8 cores (data parallel). Each
# core computes, for its shard, the transposed activations hT=[hid, inst]
# (x is pre-transposed on the host so every matmul contracts over the
# partition dim with no on-device transposes), the gated-attention scores
# s, unnormalized softmax weights w = exp(s + bc) (s is O(1) here so no
# max-subtraction is needed for stability), and the weighted pooling
# partial p = sum_i w_i * h_i. The host then reduces the 8 partial
# (p, sum w) pairs and runs the tiny text/classifier tail in numpy.
#
# Matmuls run as float32r (fp32 storage, reduced-precision PE mode at
# 1 cycle/row for moving free dim >= 256; measured end-to-end rel err
# ~6e-7 on this net). The BIR verifier requires every producer feeding
# an fp32r matmul to emit fp32r, so matmul-facing DRAM params and SBUF
# tiles are declared float32r; other readers use .bitcast(f32) views.
#
# Perf structure:
#  - The sigmoid branch is computed as 0.5 + 0.5*tanh(x/2) so ACT uses
#    only {relu, tanh, exp}, which share one activation-table set (no
#    2.7us table switches). The 0.5 factors are folded into bb and Wc
#    on the host: g = 0.5*(tanh(0.5 h Wb + 0.5 bb) + 1), and Wc is
#    pre-scaled by 0.5, so ag = a * (tanh' + 1) needs one extra DVE add.
#  - Each group's serial tail (score matmul -> exp -> broadcast ->
#    pooling) is deferred by one group, so the PE can run the next
#    group's dense matmuls instead of stalling on the ACT/DVE chain.
#  - Fused-weight-load (4-byte) matmuls can encode only ONE sync wait;
#    standalone bf16-viewed ldweights act as PE-side wait absorbers
#    placed just-in-time (not up front, which would stall the PE on all
#    weight DMAs at startup).

import numpy as np
from contextlib import ExitStack

import concourse.bass as bass
import concourse.bacc as bacc
import concourse.mybir as mybir
import concourse.tile as tile
from concourse.bass_utils import run_bass_kernel_spmd

P = 128
S = 512              # instances per group (moving free dim of every matmul)
G = 13               # groups per core
NPC = G * S          # instances per core (6656, padded)
NCORES = 8
N = 50000
FEAT, HID, DATT, TXT, NCLS = 2048, 1024, 512, 768, 2
KF, JH, DC = FEAT // P, HID // P, DATT // P   # 16, 8, 4

f32 = mybir.dt.float32
f32r = mybir.dt.float32r
AF = mybir.ActivationFunctionType
ALU = mybir.AluOpType


def build_nc(groups=G):
    # Bacc (not bare Bass): its compile() runs move_matmul_waits_to_ldweights
    # and generate_event_semaphores, which legalize instructions down to the
    # 1-sync-wait hardware limit.
    nc = bacc.Bacc("TRN2", target_bir_lowering=False, debug=False)
    npc = groups * S

    xT = nc.dram_tensor("xT", [FEAT, npc], f32r, kind="ExternalInput").ap()
    W1 = nc.dram_tensor("W1", [FEAT, HID], f32r, kind="ExternalInput").ap()
    Wa = nc.dram_tensor("Wa", [HID, DATT], f32r, kind="ExternalInput").ap()
    Wb = nc.dram_tensor("Wb", [HID, DATT], f32r, kind="ExternalInput").ap()
    Wcp = nc.dram_tensor("Wcp", [P, DC], f32r, kind="ExternalInput").ap()
    b1p = nc.dram_tensor("b1p", [P, JH], f32, kind="ExternalInput").ap()
    bap = nc.dram_tensor("bap", [P, DC], f32, kind="ExternalInput").ap()
    bbp = nc.dram_tensor("bbp", [P, DC], f32, kind="ExternalInput").ap()
    bcp = nc.dram_tensor("bcp", [1, 1], f32, kind="ExternalInput").ap()
    # mask: 1.0 for real instances, 0.0 for padding (multiplies w)
    mask = nc.dram_tensor("mask", [1, npc], f32r, kind="ExternalInput").ap()
    onesv = nc.dram_tensor("onesv", [1, P], f32r, kind="ExternalInput").ap()

    p_out = nc.dram_tensor("p_out", [P, JH], f32, kind="ExternalOutput").ap()
    w_out = nc.dram_tensor("w_out", [1, npc], f32r, kind="ExternalOutput").ap()

    def absorb(ap):
        nc.tensor.ldweights(ap.bitcast(mybir.dt.bfloat16))

    with tile.TileContext(nc) as tc, ExitStack() as ctx:
        wp = ctx.enter_context(tc.tile_pool(name="weights", bufs=1))
        xp = ctx.enter_context(tc.tile_pool(name="x", bufs=5))
        hp = ctx.enter_context(tc.tile_pool(name="h", bufs=2))
        agp = ctx.enter_context(tc.tile_pool(name="ag", bufs=2))
        sp = ctx.enter_context(tc.tile_pool(name="scratch", bufs=2))
        pp = ctx.enter_context(tc.tile_pool(name="psum", bufs=8, space="PSUM"))

        # Resident weights/biases.
        W1s = wp.tile([P, KF, HID], f32r)
        for k in range(KF):
            nc.sync.dma_start(W1s[:, k, :], W1[k * P:(k + 1) * P, :])
        Was = wp.tile([P, JH, DATT], f32r)
        Wbs = wp.tile([P, JH, DATT], f32r)
        for j in range(JH):
            nc.sync.dma_start(Was[:, j, :], Wa[j * P:(j + 1) * P, :])
            nc.sync.dma_start(Wbs[:, j, :], Wb[j * P:(j + 1) * P, :])
        Wcs = wp.tile([P, DC], f32r)
        nc.sync.dma_start(Wcs[:], Wcp)
        b1s = wp.tile([P, JH], f32)
        nc.sync.dma_start(b1s[:], b1p)
        bas = wp.tile([P, DC], f32)
        nc.sync.dma_start(bas[:], bap)
        bbs = wp.tile([P, DC], f32)
        nc.sync.dma_start(bbs[:], bbp)
        bcs = wp.tile([1, 1], f32)
        nc.sync.dma_start(bcs[:], bcp)
        ones = wp.tile([1, P], f32r)
        nc.sync.dma_start(ones[:], onesv)

        pacc = wp.tile([P, JH * groups], f32)  # per-group pooling partials
        Psb = wp.tile([P, JH], f32)

        # State carried from head(g) to the deferred tail(g).
        state = {}

        def head(g):
            isl = slice(g * S, (g + 1) * S)
            # step 1: hT[j*P+p, i] = relu(sum_f W1[f, hid] xT[f, i] + b1)
            ph = [pp.tile([P, S], f32, tag="ps", name=f"ph{g}_{j}")
                  for j in range(JH)]
            for k in range(KF):
                xt = xp.tile([P, S], f32r, tag="xt")
                nc.sync.dma_start(xt[:], xT[k * P:(k + 1) * P, isl])
                if k == 0:
                    # j=0's matmul carries a PSUM-slot wait; absorb the
                    # xt DMA wait separately so it never needs two.
                    absorb(xt[:, 0:1])
                if g == 0:
                    # group 0 only: absorb this W1 slice's DMA wait too
                    absorb(W1s[:, k, 0:1])
                for j in range(JH):
                    nc.tensor.matmul(
                        ph[j][:], W1s[:, k, j * P:(j + 1) * P], xt[:],
                        start=(k == 0), stop=(k == KF - 1))
            hT = hp.tile([P, JH, S], f32r, tag="hT")
            for j in range(JH):
                nc.scalar.activation(hT[:, j, :], ph[j][:], AF.Relu,
                                     bias=b1s[:, j:j + 1])

            # steps 2/3: aT = tanh(. + ba); tT = tanh(0.5 . + 0.5 bb);
            # agT = aT * (tT + 1)   (the 0.5s are folded into bb and Wc)
            agT = agp.tile([P, DC, S], f32r, tag="agT")
            for d in range(DC):
                pa = pp.tile([P, S], f32, tag="ps", name=f"pa{g}_{d}")
                pg = pp.tile([P, S], f32, tag="ps", name=f"pg{g}_{d}")
                if g == 0 and d == 0:
                    for j in range(JH):
                        absorb(Was[:, j, 0:1])
                        absorb(Wbs[:, j, 0:1])
                for j in range(JH):
                    nc.tensor.matmul(
                        pa[:], Was[:, j, d * P:(d + 1) * P], hT[:, j, :],
                        start=(j == 0), stop=(j == JH - 1))
                for j in range(JH):
                    nc.tensor.matmul(
                        pg[:], Wbs[:, j, d * P:(d + 1) * P], hT[:, j, :],
                        start=(j == 0), stop=(j == JH - 1))
                at = sp.tile([P, S], f32, tag="at")
                gt = sp.tile([P, S], f32, tag="gt")
                nc.scalar.activation(at[:], pa[:], AF.Tanh, bias=bas[:, d:d + 1])
                nc.scalar.activation(gt[:], pg[:], AF.Tanh,
                                     bias=bbs[:, d:d + 1], scale=0.5)
                nc.vector.tensor_scalar_add(gt[:], gt[:], 1.0)
                nc.vector.tensor_mul(agT[:, d, :], at[:], gt[:])
            state[g] = (hT, agT, isl)

        def tail(g):
            hT, agT, isl = state.pop(g)
            # step 4: s[1, i] = sum_d agT[d, i] Wc'[d]; w = exp(s+bc)*mask
            ps = pp.tile([1, S], f32, tag="ps", name=f"ps_s{g}")
            if g == 0:
                absorb(Wcs[:, 0:1])
            absorb(agT[:, 0, 0:1])
            for c in range(DC):
                nc.tensor.matmul(ps[:], Wcs[:, c:c + 1], agT[:, c, :],
                                 start=(c == 0), stop=(c == DC - 1))
            mg = sp.tile([1, S], f32r, tag="mg")
            nc.sync.dma_start(mg[:], mask[:, isl])
            wg = sp.tile([1, S], f32r, tag="wg")
            nc.scalar.activation(wg[:], ps[0:1, :], AF.Exp, bias=bcs[:, 0:1])
            wgm = sp.tile([1, S], f32r, tag="wgm")
            nc.vector.tensor_mul(wgm[:], wg[:], mg[:])
            nc.sync.dma_start(w_out[:, isl], wgm[:])

            # step 5: pooling partial p_jg[p] = sum_i hT[p,j,i] * w[i].
            # Broadcast w across partitions with a K=1 PE matmul
            # (out[m, n] = ones[0, m] * w[0, n]); DVE reads it from PSUM.
            if g == 0:
                absorb(ones[0:1, 0:1])
            absorb(wgm[0:1, 0:1])
            wB = pp.tile([P, S], f32, tag="ps", name=f"wB{g}")
            nc.tensor.matmul(wB[:], ones[:], wgm[:], start=True, stop=True)
            for j in range(JH):
                wh = sp.tile([P, S], f32, tag="wh")
                nc.vector.tensor_mul(wh[:], hT[:, j, :].bitcast(f32), wB[:])
                nc.vector.tensor_reduce(
                    pacc[:, j * groups + g:j * groups + g + 1], wh[:],
                    axis=mybir.AxisListType.X, op=ALU.add)

        # Software pipeline: tail(g-1) is emitted after head(g), so the
        # PE interleaves g-1's short dependent matmuls with g's dense work.
        for g in range(groups):
            head(g)
            if g >= 1:
                tail(g - 1)
        tail(groups - 1)

        nc.vector.tensor_reduce(
            Psb[:], pacc[:].rearrange("p (j g) -> p j g", g=groups),
            axis=mybir.AxisListType.X, op=ALU.add)
        nc.sync.dma_start(p_out, Psb[:])

    nc.compile()
    return nc


_CACHE = {}


def _get_nc():
    if "nc" not in _CACHE:
        _CACHE["nc"] = build_nc()
    return _CACHE["nc"]


def make_in_maps(x, W1, b1, Wa, ba, Wb, bb, Wc, bc):
    """Shard/pack the full inputs into per-core input maps."""
    x = np.ascontiguousarray(np.asarray(x, np.float32))
    ntot = NCORES * NPC
    xTp = np.zeros((FEAT, ntot), np.float32)
    xTp[:, :N] = x.T
    m = np.zeros(ntot, np.float32)
    m[:N] = 1.0

    W1 = np.ascontiguousarray(np.asarray(W1, np.float32))
    Wa = np.ascontiguousarray(np.asarray(Wa, np.float32))
    Wb = np.ascontiguousarray(np.asarray(Wb, np.float32))
    # sigmoid(x) = 0.5 + 0.5*tanh(x/2): fold 0.5 into bb (tanh bias) and
    # 0.5 into Wc (ag = a*(tanh'+1) on device).
    Wcp = np.ascontiguousarray(
        (np.asarray(Wc, np.float32)[:, 0] * 0.5).reshape(DC, P).T)
    b1p = np.ascontiguousarray(np.asarray(b1, np.float32).reshape(JH, P).T)
    bap = np.ascontiguousarray(np.asarray(ba, np.float32).reshape(DC, P).T)
    bbp = np.ascontiguousarray(
        (np.asarray(bb, np.float32) * 0.5).reshape(DC, P).T)
    bcp = np.asarray(bc, np.float32).reshape(1, 1)

    in_maps = []
    for c in range(NCORES):
        sl = slice(c * NPC, (c + 1) * NPC)
        in_maps.append({
            "xT": np.ascontiguousarray(xTp[:, sl]),
            "W1": W1, "Wa": Wa, "Wb": Wb, "Wcp": Wcp,
            "b1p": b1p, "bap": bap, "bbp": bbp, "bcp": bcp,
            "mask": np.ascontiguousarray(m[sl].reshape(1, NPC)),
            "onesv": np.ones((1, P), np.float32),
        })
    return in_maps


def finish(results, text_emb, Wt, bt, Wcls, bcls):
    """Host-side reduction of per-core partials + text/classifier tail."""
    p = np.zeros(HID, np.float64)
    z = 0.0
    for r in results:
        p += r["p_out"].astype(np.float64).T.reshape(-1)
        z += float(r["w_out"].sum(dtype=np.float64))
    M = (p / z).astype(np.float32)[None, :]                    # [1, 1024]
    text_emb = np.asarray(text_emb, np.float32)
    t = np.maximum(text_emb @ np.asarray(Wt, np.float32)
                   + np.asarray(bt, np.float32), 0.0)
    logits = (M + t) @ np.asarray(Wcls, np.float32) + np.asarray(bcls, np.float32)
    return logits.astype(np.float32)


def kernel(x, text_emb, W1, b1, Wa, ba, Wb, bb, Wc, bc, Wt, bt, Wcls, bcls,
           **kw):
    in_maps = make_in_maps(x, W1, b1, Wa, ba, Wb, bb, Wc, bc)
    res = run_bass_kernel_spmd(_get_nc(), in_maps, core_ids=list(range(NCORES)),
                               **kw)
    return finish(res.results, text_emb, Wt, bt, Wcls, bcls)


# revision 24
# speedup vs baseline: 1.1545x; 1.0980x over previous
# CHIEF attention-MIL pooling kernel for 8 TRN2 NeuronCores.
#
# Reference computation (N=50000, FEAT=2048, HID=1024, D_ATT=512):
#   h = relu(x @ W1 + b1)            [N, 1024]
#   a = tanh(h @ Wa + ba)            [N, 512]
#   g = sigmoid(h @ Wb + bb)         [N, 512]
#   s = (a*g) @ Wc + bc              [N, 1]
#   A = softmax(s.T)                 [1, N]
#   M = A @ h + relu(text @ Wt + bt) [1, 1024]
#   logits = M @ Wcls + bcls         [1, 2]
#
# Strategy: shard the instance dim N across 8 cores (data parallel). Each
# core computes, for its shard, the transposed activations hT=[hid, inst]
# (x is pre-transposed on the host so every matmul contracts over the
# partition dim with no on-device transposes), the gated-attention scores
# s, unnormalized softmax weights w = exp(s + bc) (s is O(1) here so no
# max-subtraction is needed for stability), and the weighted pooling
# partial p = sum_i w_i * h_i. The host then reduces the 8 partial
# (p, sum w) pairs and runs the tiny text/classifier tail in numpy.
#
# Matmuls run as float32r (fp32 storage, reduced-precision PE mode at
# 1 cycle/row for moving free dim >= 256; measured end-to-end rel err
# ~6e-7 on this net). The BIR verifier requires every producer feeding
# an fp32r matmul to emit fp32r, so matmul-facing DRAM params and SBUF
# tiles are declared float32r; other readers use .bitcast(f32) views.
#
# Perf structure:
#  - The sigmoid branch is computed as 0.5 + 0.5*tanh(x/2) so ACT uses
#    only {relu, tanh, exp}, which share one activation-table set (no
#    2.7us table switches). The 0.5 factors are folded into bb and Wc
#    on the host: g = 0.5*(tanh(0.5 h Wb + 0.5 bb) + 1), and Wc is
#    pre-scaled by 0.5, so ag = a * (tanh' + 1) needs one extra DVE add.
#  - Each group's serial tail (score matmul -> exp -> broadcast ->
#    pooling) is deferred by one group, so the PE can run the next
#    group's dense matmuls instead of stalling on the ACT/DVE chain.
#  - Fused-weight-load (4-byte) matmuls can encode only ONE sync wait;
#    standalone bf16-viewed ldweights act as PE-side wait absorbers
#    placed just-in-time (not up front, which would stall the PE on all
#    weight DMAs at startup).

import numpy as np
from contextlib import ExitStack

import concourse.bass as bass
import concourse.bacc as bacc
import concourse.mybir as mybir
import concourse.tile as tile
from concourse.bass_utils import run_bass_kernel_spmd

P = 128
S = 512              # instances per group (moving free dim of every matmul)
G = 13               # groups per core
NPC = G * S          # instances per core (6656, padded)
NCORES = 8
N = 50000
FEAT, HID, DATT, TXT, NCLS = 2048, 1024, 512, 768, 2
KF, JH, DC = FEAT // P, HID // P, DATT // P   # 16, 8, 4

f32 = mybir.dt.float32
f32r = mybir.dt.float32r
AF = mybir.ActivationFunctionType
ALU = mybir.AluOpType


def build_nc(groups=G):
    # Bacc (not bare Bass): its compile() runs move_matmul_waits_to_ldweights
    # and generate_event_semaphores, which legalize instructions down to the
    # 1-sync-wait hardware limit.
    nc = bacc.Bacc("TRN2", target_bir_lowering=False, debug=False)
    npc = groups * S

    xT = nc.dram_tensor("xT", [FEAT, npc], f32r, kind="ExternalInput").ap()
    W1 = nc.dram_tensor("W1", [FEAT, HID], f32r, kind="ExternalInput").ap()
    Wa = nc.dram_tensor("Wa", [HID, DATT], f32r, kind="ExternalInput").ap()
    Wb = nc.dram_tensor("Wb", [HID, DATT], f32r, kind="ExternalInput").ap()
    Wcp = nc.dram_tensor("Wcp", [P, DC], f32r, kind="ExternalInput").ap()
    b1p = nc.dram_tensor("b1p", [P, JH], f32, kind="ExternalInput").ap()
    bap = nc.dram_tensor("bap", [P, DC], f32, kind="ExternalInput").ap()
    bbp = nc.dram_tensor("bbp", [P, DC], f32, kind="ExternalInput").ap()
    bcp = nc.dram_tensor("bcp", [1, 1], f32, kind="ExternalInput").ap()
    # mask: 1.0 for real instances, 0.0 for padding (multiplies w)
    mask = nc.dram_tensor("mask", [1, npc], f32r, kind="ExternalInput").ap()
    onesv = nc.dram_tensor("onesv", [1, P], f32r, kind="ExternalInput").ap()

    p_out = nc.dram_tensor("p_out", [P, JH], f32, kind="ExternalOutput").ap()
    w_out = nc.dram_tensor("w_out", [1, npc], f32r, kind="ExternalOutput").ap()

    def absorb(ap):
        nc.tensor.ldweights(ap.bitcast(mybir.dt.bfloat16))

    with tile.TileContext(nc) as tc, ExitStack() as ctx:
        wp = ctx.enter_context(tc.tile_pool(name="weights", bufs=1))
        xp = ctx.enter_context(tc.tile_pool(name="x", bufs=12))
        hp = ctx.enter_context(tc.tile_pool(name="h", bufs=2))
        agp = ctx.enter_context(tc.tile_pool(name="ag", bufs=2))
        sp = ctx.enter_context(tc.tile_pool(name="scratch", bufs=2))
        pp = ctx.enter_context(tc.tile_pool(name="psum", bufs=8, space="PSUM"))

        # Resident weights/biases.
        W1s = wp.tile([P, KF, HID], f32r)
        for k in range(KF):
            nc.sync.dma_start(W1s[:, k, :], W1[k * P:(k + 1) * P, :])
        Was = wp.tile([P, JH, DATT], f32r)
        Wbs = wp.tile([P, JH, DATT], f32r)
        for j in range(JH):
            nc.sync.dma_start(Was[:, j, :], Wa[j * P:(j + 1) * P, :])
            nc.sync.dma_start(Wbs[:, j, :], Wb[j * P:(j + 1) * P, :])
        Wcs = wp.tile([P, DC], f32r)
        nc.sync.dma_start(Wcs[:], Wcp)
        b1s = wp.tile([P, JH], f32)
        nc.sync.dma_start(b1s[:], b1p)
        bas = wp.tile([P, DC], f32)
        nc.sync.dma_start(bas[:], bap)
        bbs = wp.tile([P, DC], f32)
        nc.sync.dma_start(bbs[:], bbp)
        bcs = wp.tile([1, 1], f32)
        nc.sync.dma_start(bcs[:], bcp)
        ones = wp.tile([1, P], f32r)
        nc.sync.dma_start(ones[:], onesv)

        pacc = wp.tile([P, JH * groups], f32)  # per-group pooling partials
        Psb = wp.tile([P, JH], f32)

        # State carried from head(g) to the deferred tail(g).
        state = {}

        def head(g):
            isl = slice(g * S, (g + 1) * S)
            # step 1: hT[j*P+p, i] = relu(sum_f W1[f, hid] xT[f, i] + b1)
            ph = [pp.tile([P, S], f32, tag="ps", name=f"ph{g}_{j}")
                  for j in range(JH)]
            for k in range(KF):
                xt = xp.tile([P, S], f32r, tag="xt")
                nc.sync.dma_start(xt[:], xT[k * P:(k + 1) * P, isl])
                if k == 0:
                    # j=0's matmul carries a PSUM-slot wait; absorb the
                    # xt DMA wait separately so it never needs two.
                    absorb(xt[:, 0:1])
                if g == 0:
                    # group 0 only: absorb this W1 slice's DMA wait too
                    absorb(W1s[:, k, 0:1])
                for j in range(JH):
                    nc.tensor.matmul(
                        ph[j][:], W1s[:, k, j * P:(j + 1) * P], xt[:],
                        start=(k == 0), stop=(k == KF - 1))
            hT = hp.tile([P, JH, S], f32r, tag="hT")
            for j in range(JH):
                nc.scalar.activation(hT[:, j, :], ph[j][:], AF.Relu,
                                     bias=b1s[:, j:j + 1])

            # steps 2/3: aT = tanh(. + ba); tT = tanh(0.5 . + 0.5 bb);
            # agT = aT * (tT + 1)   (the 0.5s are folded into bb and Wc)
            agT = agp.tile([P, DC, S], f32r, tag="agT")
            for d in range(DC):
                pa = pp.tile([P, S], f32, tag="ps", name=f"pa{g}_{d}")
                pg = pp.tile([P, S], f32, tag="ps", name=f"pg{g}_{d}")
                if g == 0 and d == 0:
                    for j in range(JH):
                        absorb(Was[:, j, 0:1])
                        absorb(Wbs[:, j, 0:1])
                for j in range(JH):
                    nc.tensor.matmul(
                        pa[:], Was[:, j, d * P:(d + 1) * P], hT[:, j, :],
                        start=(j == 0), stop=(j == JH - 1))
                for j in range(JH):
                    nc.tensor.matmul(
                        pg[:], Wbs[:, j, d * P:(d + 1) * P], hT[:, j, :],
                        start=(j == 0), stop=(j == JH - 1))
                at = sp.tile([P, S], f32, tag="at")
                gt = sp.tile([P, S], f32, tag="gt")
                nc.scalar.activation(at[:], pa[:], AF.Tanh, bias=bas[:, d:d + 1])
                nc.scalar.activation(gt[:], pg[:], AF.Tanh,
                                     bias=bbs[:, d:d + 1], scale=0.5)
                nc.vector.tensor_scalar_add(gt[:], gt[:], 1.0)
                nc.vector.tensor_mul(agT[:, d, :], at[:], gt[:])
            state[g] = (hT, agT, isl)

        def tail(g):
            hT, agT, isl = state.pop(g)
            # step 4: s[1, i] = sum_d agT[d, i] Wc'[d]; w = exp(s+bc)*mask
            ps = pp.tile([1, S], f32, tag="ps", name=f"ps_s{g}")
            if g == 0:
                absorb(Wcs[:, 0:1])
            absorb(agT[:, 0, 0:1])
            for c in range(DC):
                nc.tensor.matmul(ps[:], Wcs[:, c:c + 1], agT[:, c, :],
                                 start=(c == 0), stop=(c == DC - 1))
            mg = sp.tile([1, S], f32r, tag="mg")
            nc.sync.dma_start(mg[:], mask[:, isl])
            wg = sp.tile([1, S], f32r, tag="wg")
            nc.scalar.activation(wg[:], ps[0:1, :], AF.Exp, bias=bcs[:, 0:1])
            wgm = sp.tile([1, S], f32r, tag="wgm")
            nc.vector.tensor_mul(wgm[:], wg[:], mg[:])
            nc.sync.dma_start(w_out[:, isl], wgm[:])

            # step 5: pooling partial p_jg[p] = sum_i hT[p,j,i] * w[i].
            # Broadcast w across partitions with a K=1 PE matmul
            # (out[m, n] = ones[0, m] * w[0, n]); DVE reads it from PSUM.
            if g == 0:
                absorb(ones[0:1, 0:1])
            absorb(wgm[0:1, 0:1])
            wB = pp.tile([P, S], f32, tag="ps", name=f"wB{g}")
            nc.tensor.matmul(wB[:], ones[:], wgm[:], start=True, stop=True)
            for j in range(JH):
                wh = sp.tile([P, S], f32, tag="wh")
                nc.vector.tensor_mul(wh[:], hT[:, j, :].bitcast(f32), wB[:])
                nc.vector.tensor_reduce(
                    pacc[:, j * groups + g:j * groups + g + 1], wh[:],
                    axis=mybir.AxisListType.X, op=ALU.add)

        # Software pipeline: tail(g-1) is emitted after head(g), so the
        # PE interleaves g-1's short dependent matmuls with g's dense work.
        for g in range(groups):
            head(g)
            if g >= 1:
                tail(g - 1)
        tail(groups - 1)

        nc.vector.tensor_reduce(
            Psb[:], pacc[:].rearrange("p (j g) -> p j g", g=groups),
            axis=mybir.AxisListType.X, op=ALU.add)
        nc.sync.dma_start(p_out, Psb[:])

    nc.compile()
    return nc


_CACHE = {}


def _get_nc():
    if "nc" not in _CACHE:
        _CACHE["nc"] = build_nc()
    return _CACHE["nc"]


def make_in_maps(x, W1, b1, Wa, ba, Wb, bb, Wc, bc):
    """Shard/pack the full inputs into per-core input maps."""
    x = np.ascontiguousarray(np.asarray(x, np.float32))
    ntot = NCORES * NPC
    xTp = np.zeros((FEAT, ntot), np.float32)
    xTp[:, :N] = x.T
    m = np.zeros(ntot, np.float32)
    m[:N] = 1.0

    W1 = np.ascontiguousarray(np.asarray(W1, np.float32))
    Wa = np.ascontiguousarray(np.asarray(Wa, np.float32))
    Wb = np.ascontiguousarray(np.asarray(Wb, np.float32))
    # sigmoid(x) = 0.5 + 0.5*tanh(x/2): fold 0.5 into bb (tanh bias) and
    # 0.5 into Wc (ag = a*(tanh'+1) on device).
    Wcp = np.ascontiguousarray(
        (np.asarray(Wc, np.float32)[:, 0] * 0.5).reshape(DC, P).T)
    b1p = np.ascontiguousarray(np.asarray(b1, np.float32).reshape(JH, P).T)
    bap = np.ascontiguousarray(np.asarray(ba, np.float32).reshape(DC, P).T)
    bbp = np.ascontiguousarray(
        (np.asarray(bb, np.float32) * 0.5).reshape(DC, P).T)
    bcp = np.asarray(bc, np.float32).reshape(1, 1)

    in_maps = []
    for c in range(NCORES):
        sl = slice(c * NPC, (c + 1) * NPC)
        in_maps.append({
            "xT": np.ascontiguousarray(xTp[:, sl]),
            "W1": W1, "Wa": Wa, "Wb": Wb, "Wcp": Wcp,
            "b1p": b1p, "bap": bap, "bbp": bbp, "bcp": bcp,
            "mask": np.ascontiguousarray(m[sl].reshape(1, NPC)),
            "onesv": np.ones((1, P), np.float32),
        })
    return in_maps


def finish(results, text_emb, Wt, bt, Wcls, bcls):
    """Host-side reduction of per-core partials + text/classifier tail."""
    p = np.zeros(HID, np.float64)
    z = 0.0
    for r in results:
        p += r["p_out"].astype(np.float64).T.reshape(-1)
        z += float(r["w_out"].sum(dtype=np.float64))
    M = (p / z).astype(np.float32)[None, :]                    # [1, 1024]
    text_emb = np.asarray(text_emb, np.float32)
    t = np.maximum(text_emb @ np.asarray(Wt, np.float32)
                   + np.asarray(bt, np.float32), 0.0)
    logits = (M + t) @ np.asarray(Wcls, np.float32) + np.asarray(bcls, np.float32)
    return logits.astype(np.float32)


def kernel(x, text_emb, W1, b1, Wa, ba, Wb, bb, Wc, bc, Wt, bt, Wcls, bcls,
           **kw):
    in_maps = make_in_maps(x, W1, b1, Wa, ba, Wb, bb, Wc, bc)
    res = run_bass_kernel_spmd(_get_nc(), in_maps, core_ids=list(range(NCORES)),
                               **kw)
    return finish(res.results, text_emb, Wt, bt, Wcls, bcls)


# revision 29
# speedup vs baseline: 1.2244x; 1.0605x over previous
# CHIEF attention-MIL pooling kernel for 8 TRN2 NeuronCores.
#
# Reference computation (N=50000, FEAT=2048, HID=1024, D_ATT=512):
#   h = relu(x @ W1 + b1)            [N, 1024]
#   a = tanh(h @ Wa + ba)            [N, 512]
#   g = sigmoid(h @ Wb + bb)         [N, 512]
#   s = (a*g) @ Wc + bc              [N, 1]
#   A = softmax(s.T)                 [1, N]
#   M = A @ h + relu(text @ Wt + bt) [1, 1024]
#   logits = M @ Wcls + bcls         [1, 2]
#
# Strategy: shard the instance dim N across 8 cores (data parallel). Each
# core computes, for its shard, the transposed activations hT=[hid, inst]
# (x is pre-transposed on the host so every matmul contracts over the
# partition dim with no on-device transposes), the gated-attention scores
# s, unnormalized softmax weights w = exp(s + bc) (s is O(1) here so no
# max-subtraction is needed for stability), and the weighted pooling
# partial p = sum_i w_i * h_i. The host then reduces the 8 partial
# (p, sum w) pairs and runs the tiny text/classifier tail in numpy.
#
# Matmuls run as float32r (fp32 storage, reduced-precision PE mode at
# 1 cycle/row for moving free dim >= 256; measured end-to-end rel err
# ~6e-7 on this net). The BIR verifier requires every producer feeding
# an fp32r matmul to emit fp32r, so matmul-facing DRAM params and SBUF
# tiles are declared float32r; other readers use .bitcast(f32) views.
#
# Perf structure:
#  - The sigmoid branch is computed as 0.5 + 0.5*tanh(x/2) so ACT uses
#    only {relu, tanh, exp}, which share one activation-table set (no
#    2.7us table switches). The 0.5 factors are folded into bb and Wc
#    on the host: g = 0.5*(tanh(0.5 h Wb + 0.5 bb) + 1), and Wc is
#    pre-scaled by 0.5, so ag = a * (tanh' + 1) needs one extra DVE add.
#  - Each group's serial tail (score matmul -> exp -> broadcast ->
#    pooling) is deferred by one group, so the PE can run the next
#    group's dense matmuls instead of stalling on the ACT/DVE chain.
#  - Fused-weight-load (4-byte) matmuls can encode only ONE sync wait;
#    standalone bf16-viewed ldweights act as PE-side wait absorbers
#    placed just-in-time (not up front, which would stall the PE on all
#    weight DMAs at startup).

import numpy as np
from contextlib import ExitStack

import concourse.bass as bass
import concourse.bacc as bacc
import concourse.mybir as mybir
import concourse.tile as tile
from concourse.bass_utils import run_bass_kernel_spmd

P = 128
S = 512              # instances per group (moving free dim of every matmul)
G = 13               # groups per core
NPC = G * S          # instances per core (6656, padded)
NCORES = 8
N = 50000
FEAT, HID, DATT, TXT, NCLS = 2048, 1024, 512, 768, 2
KF, JH, DC = FEAT // P, HID // P, DATT // P   # 16, 8, 4

f32 = mybir.dt.float32
f32r = mybir.dt.float32r
AF = mybir.ActivationFunctionType
ALU = mybir.AluOpType


def build_nc(groups=G):
    # Bacc (not bare Bass): its compile() runs move_matmul_waits_to_ldweights
    # and generate_event_semaphores, which legalize instructions down to the
    # 1-sync-wait hardware limit.
    nc = bacc.Bacc("TRN2", target_bir_lowering=False, debug=False)
    npc = groups * S

    xT = nc.dram_tensor("xT", [FEAT, npc], f32r, kind="ExternalInput").ap()
    W1 = nc.dram_tensor("W1", [FEAT, HID], f32r, kind="ExternalInput").ap()
    Wa = nc.dram_tensor("Wa", [HID, DATT], f32r, kind="ExternalInput").ap()
    Wb = nc.dram_tensor("Wb", [HID, DATT], f32r, kind="ExternalInput").ap()
    Wcp = nc.dram_tensor("Wcp", [P, DC], f32r, kind="ExternalInput").ap()
    b1p = nc.dram_tensor("b1p", [P, JH], f32, kind="ExternalInput").ap()
    bap = nc.dram_tensor("bap", [P, DC], f32, kind="ExternalInput").ap()
    bbp = nc.dram_tensor("bbp", [P, DC], f32, kind="ExternalInput").ap()
    bcp = nc.dram_tensor("bcp", [1, 1], f32, kind="ExternalInput").ap()
    # mask: 1.0 for real instances, 0.0 for padding (multiplies w)
    mask = nc.dram_tensor("mask", [1, npc], f32r, kind="ExternalInput").ap()

    p_out = nc.dram_tensor("p_out", [P, JH], f32, kind="ExternalOutput").ap()
    w_out = nc.dram_tensor("w_out", [1, npc], f32r, kind="ExternalOutput").ap()

    def absorb(ap):
        nc.tensor.ldweights(ap.bitcast(mybir.dt.bfloat16))

    with tile.TileContext(nc) as tc, ExitStack() as ctx:
        wp = ctx.enter_context(tc.tile_pool(name="weights", bufs=1))
        xp = ctx.enter_context(tc.tile_pool(name="x", bufs=12))
        hp = ctx.enter_context(tc.tile_pool(name="h", bufs=2))
        agp = ctx.enter_context(tc.tile_pool(name="ag", bufs=2))
        sp = ctx.enter_context(tc.tile_pool(name="scratch", bufs=2))
        pp = ctx.enter_context(tc.tile_pool(name="psum", bufs=8, space="PSUM"))

        # Resident weights/biases. W1 slices are DMA'd up front (needed
        # immediately); Wa/Wb/Wc DMAs are deferred to first use so they
        # don't delay group 0's xt streaming on the shared HWDGE queues.
        W1s = wp.tile([P, KF, HID], f32r)
        for k in range(KF):
            nc.sync.dma_start(W1s[:, k, :], W1[k * P:(k + 1) * P, :])
        Was = wp.tile([P, JH, DATT], f32r)
        Wbs = wp.tile([P, JH, DATT], f32r)
        Wcs = wp.tile([P, DC], f32r)
        b1s = wp.tile([P, JH], f32)
        nc.sync.dma_start(b1s[:], b1p)
        bas = wp.tile([P, DC], f32)
        nc.sync.dma_start(bas[:], bap)
        bbs = wp.tile([P, DC], f32)
        nc.sync.dma_start(bbs[:], bbp)
        bcs = wp.tile([1, 1], f32)
        nc.sync.dma_start(bcs[:], bcp)

        pacc = wp.tile([P, JH * groups], f32)  # per-group pooling partials
        Psb = wp.tile([P, JH], f32)

        # State carried from head(g) to the deferred tail(g).
        state = {}

        def head(g):
            isl = slice(g * S, (g + 1) * S)
            # step 1: hT[j*P+p, i] = relu(sum_f W1[f, hid] xT[f, i] + b1)
            ph = [pp.tile([P, S], f32, tag="ps", name=f"ph{g}_{j}")
                  for j in range(JH)]
            for k in range(KF):
                xt = xp.tile([P, S], f32r, tag="xt")
                nc.sync.dma_start(xt[:], xT[k * P:(k + 1) * P, isl])
                if k == 0:
                    # j=0's matmul carries a PSUM-slot wait; absorb the
                    # xt DMA wait separately so it never needs two.
                    absorb(xt[:, 0:1])
                if g == 0:
                    # group 0 only: absorb this W1 slice's DMA wait too
                    absorb(W1s[:, k, 0:1])
                for j in range(JH):
                    nc.tensor.matmul(
                        ph[j][:], W1s[:, k, j * P:(j + 1) * P], xt[:],
                        start=(k == 0), stop=(k == KF - 1))
            hT = hp.tile([P, JH, S], f32r, tag="hT")
            for j in range(JH):
                nc.scalar.activation(hT[:, j, :], ph[j][:], AF.Relu,
                                     bias=b1s[:, j:j + 1])

            # steps 2/3: aT = tanh(. + ba); tT = tanh(0.5 . + 0.5 bb);
            # agT = aT * (tT + 1)   (the 0.5s are folded into bb and Wc)
            agT = agp.tile([P, DC, S], f32r, tag="agT")
            for d in range(DC):
                pa = pp.tile([P, S], f32, tag="ps", name=f"pa{g}_{d}")
                pg = pp.tile([P, S], f32, tag="ps", name=f"pg{g}_{d}")
                if g == 0 and d == 0:
                    for j in range(JH):
                        nc.sync.dma_start(Was[:, j, :], Wa[j * P:(j + 1) * P, :])
                        nc.sync.dma_start(Wbs[:, j, :], Wb[j * P:(j + 1) * P, :])
                    for j in range(JH):
                        absorb(Was[:, j, 0:1])
                        absorb(Wbs[:, j, 0:1])
                for j in range(JH):
                    nc.tensor.matmul(
                        pa[:], Was[:, j, d * P:(d + 1) * P], hT[:, j, :],
                        start=(j == 0), stop=(j == JH - 1))
                for j in range(JH):
                    nc.tensor.matmul(
                        pg[:], Wbs[:, j, d * P:(d + 1) * P], hT[:, j, :],
                        start=(j == 0), stop=(j == JH - 1))
                at = sp.tile([P, S], f32, tag="at")
                gt = sp.tile([P, S], f32, tag="gt")
                nc.scalar.activation(at[:], pa[:], AF.Tanh, bias=bas[:, d:d + 1])
                nc.scalar.activation(gt[:], pg[:], AF.Tanh,
                                     bias=bbs[:, d:d + 1], scale=0.5)
                nc.vector.tensor_scalar_add(gt[:], gt[:], 1.0)
                nc.vector.tensor_mul(agT[:, d, :], at[:], gt[:])
            state[g] = (hT, agT, isl)

        def tail(g):
            hT, agT, isl = state.pop(g)
            # step 4: s[1, i] = sum_d agT[d, i] Wc'[d]; w = exp(s+bc)*mask
            ps = pp.tile([1, S], f32, tag="ps", name=f"ps_s{g}")
            if g == 0:
                nc.sync.dma_start(Wcs[:], Wcp)
                absorb(Wcs[:, 0:1])
            absorb(agT[:, 0, 0:1])
            for c in range(DC):
                nc.tensor.matmul(ps[:], Wcs[:, c:c + 1], agT[:, c, :],
                                 start=(c == 0), stop=(c == DC - 1))
            mg = sp.tile([1, S], f32r, tag="mg")
            nc.sync.dma_start(mg[:], mask[:, isl])
            wg = sp.tile([1, S], f32r, tag="wg")
            nc.scalar.activation(wg[:], ps[0:1, :], AF.Exp, bias=bcs[:, 0:1])
            wgm = sp.tile([1, S], f32r, tag="wgm")
            nc.vector.tensor_mul(wgm[:], wg[:], mg[:])
            nc.sync.dma_start(w_out[:, isl], wgm[:])

            # step 5: pooling partial p_jg[p] = sum_i hT[p,j,i] * w[i].
            # Broadcast w across partitions on the (otherwise idle) GpSimd
            # engine, keeping the PE out of the exp->mask->broadcast chain.
            wB = sp.tile([P, S], f32, tag="wB")
            nc.gpsimd.partition_broadcast(wB[:], wgm[:].bitcast(f32))
            for j in range(JH):
                wh = sp.tile([P, S], f32, tag="wh")
                nc.vector.tensor_mul(wh[:], hT[:, j, :].bitcast(f32), wB[:])
                nc.vector.tensor_reduce(
                    pacc[:, j * groups + g:j * groups + g + 1], wh[:],
                    axis=mybir.AxisListType.X, op=ALU.add)

        # Software pipeline: tail(g-1) is emitted after head(g), so the
        # PE interleaves g-1's short dependent matmuls with g's dense work.
        for g in range(groups):
            head(g)
            if g >= 1:
                tail(g - 1)
        tail(groups - 1)

        nc.vector.tensor_reduce(
            Psb[:], pacc[:].rearrange("p (j g) -> p j g", g=groups),
            axis=mybir.AxisListType.X, op=ALU.add)
        nc.sync.dma_start(p_out, Psb[:])

    nc.compile()
    return nc


_CACHE = {}


def _get_nc():
    if "nc" not in _CACHE:
        _CACHE["nc"] = build_nc()
    return _CACHE["nc"]


def make_in_maps(x, W1, b1, Wa, ba, Wb, bb, Wc, bc):
    """Shard/pack the full inputs into per-core input maps."""
    x = np.ascontiguousarray(np.asarray(x, np.float32))
    ntot = NCORES * NPC
    xTp = np.zeros((FEAT, ntot), np.float32)
    xTp[:, :N] = x.T
    m = np.zeros(ntot, np.float32)
    m[:N] = 1.0

    W1 = np.ascontiguousarray(np.asarray(W1, np.float32))
    Wa = np.ascontiguousarray(np.asarray(Wa, np.float32))
    Wb = np.ascontiguousarray(np.asarray(Wb, np.float32))
    # sigmoid(x) = 0.5 + 0.5*tanh(x/2): fold 0.5 into bb (tanh bias) and
    # 0.5 into Wc (ag = a*(tanh'+1) on device).
    Wcp = np.ascontiguousarray(
        (np.asarray(Wc, np.float32)[:, 0] * 0.5).reshape(DC, P).T)
    b1p = np.ascontiguousarray(np.asarray(b1, np.float32).reshape(JH, P).T)
    bap = np.ascontiguousarray(np.asarray(ba, np.float32).reshape(DC, P).T)
    bbp = np.ascontiguousarray(
        (np.asarray(bb, np.float32) * 0.5).reshape(DC, P).T)
    bcp = np.asarray(bc, np.float32).reshape(1, 1)

    in_maps = []
    for c in range(NCORES):
        sl = slice(c * NPC, (c + 1) * NPC)
        in_maps.append({
            "xT": np.ascontiguousarray(xTp[:, sl]),
            "W1": W1, "Wa": Wa, "Wb": Wb, "Wcp": Wcp,
            "b1p": b1p, "bap": bap, "bbp": bbp, "bcp": bcp,
            "mask": np.ascontiguousarray(m[sl].reshape(1, NPC)),
        })
    return in_maps


def finish(results, text_emb, Wt, bt, Wcls, bcls):
    """Host-side reduction of per-core partials + text/classifier tail."""
    p = np.zeros(HID, np.float64)
    z = 0.0
    for r in results:
        p += r["p_out"].astype(np.float64).T.reshape(-1)
        z += float(r["w_out"].sum(dtype=np.float64))
    M = (p / z).astype(np.float32)[None, :]                    # [1, 1024]
    text_emb = np.asarray(text_emb, np.float32)
    t = np.maximum(text_emb @ np.asarray(Wt, np.float32)
                   + np.asarray(bt, np.float32), 0.0)
    logits = (M + t) @ np.asarray(Wcls, np.float32) + np.asarray(bcls, np.float32)
    return logits.astype(np.float32)


def kernel(x, text_emb, W1, b1, Wa, ba, Wb, bb, Wc, bc, Wt, bt, Wcls, bcls,
           **kw):
    in_maps = make_in_maps(x, W1, b1, Wa, ba, Wb, bb, Wc, bc)
    res = run_bass_kernel_spmd(_get_nc(), in_maps, core_ids=list(range(NCORES)),
                               **kw)
    return finish(res.results, text_emb, Wt, bt, Wcls, bcls)
